# revision 48
# baseline (speedup 1.0000x reference)
"""MoE grouped-FFN kernel for Trainium2 (8 NeuronCores, expert-parallel).

Problem: x [1, 2048, 1024] fp32, 32 experts x 64 tokens each,
per-expert FFN 1024 -> 4096 (gelu) -> 1024.

Sharding: expert-parallel, 4 experts per core. Tokens are statically
pre-chunked per expert (dim 1 == E*C), so each core just gets its 4
experts' token rows + weights; outputs concatenate back. No collectives.

The problem is HBM-bound, so weights stream in fp8 e3m4 (1 byte/elem,
4 mantissa bits — 2x the precision of e4m3 at the same width). All
weight chunks are pre-scaled by SCALE=64 on the host so values land in
e3m4's normal range (sigma*64 ~ 1.28 vs e3m4 normals [0.25, 15.5]);
bf16 fallback chunks (n_q1/n_q2 knobs) get the same exact power-of-2
scale so every PSUM accumulation is uniformly 64-scaled regardless of
source dtype. Unscales ride existing ops: gelu's activation computes
func(in*scale + bias) with scale=1/64, and the mm2 evacuation Copy
carries scale=1/64. Activations (xT, h, out) are fp16 — same bytes as
bf16, 8x less rounding noise. Measured on HW: rel err 1.77e-2 vs the
2e-2 gate (numpy sim predicted 1.764e-2).

Dataflow is WEIGHT-STATIONARY both matmuls: the [128,128] weight tile
is the PE stationary operand and the 64 tokens stream as the moving
operand. A fresh fp8 stationary per 64-column matmul sustains ~45ns
(measured: LD_WEIGHTS mostly overlaps), so PE busy is ~93us/core vs
121us for the token-stationary form — and mm1's PSUM output lands
[f, c], which kills the PE transposes and DVE copies the
token-stationary form needed (gelu reads PSUM directly with b1 as a
per-partition bias), and mm2's PSUM lands [d, c] where b2 is a cheap
per-d rank-1 and the host un-transposes the stored output for free.

Per-core dataflow, software-pipelined over 8 uniform 512-wide chunks
per expert, both weight streams issued from the SP queue strictly
alternating w1(g+LEAD), w2(g) (single-queue emission pins the on-wire
DMA order; w1 feeds the deeper mm1->gelu->mm2 chain):
  mm1: h[128f, c] += W1[k,ftile].T @ xT[k]      (8 k-tiles chained per f-tile)
  ACT: hT[ft] = gelu(h_psum/64 + b1[f])         (PSUM read, per-partition bias)
  mm2: oT[128d, c] += W2[ft,dblk].T @ hT[ft]    (32 f-tiles chained per d-block)
  DVE evacuates oT*(1/64) + b2[d] to fp16 [128d, 8, c]; host un-transposes.
"""

import os
import numpy as np

E, C, D, F = 32, 64, 1024, 4096
N_CORES = 8
E_LOC = E // N_CORES  # experts per core
P = 128
KT1 = D // P  # 8 K-tiles in mm1; also 8 d-blocks in mm2's output
FT = F // P  # 32 f-tiles
SCALE = 64.0  # host pre-scale on all weight chunks (power of 2: exact in bf16)

DEFAULT_CFG = dict(
    n_q1=8,        # leading W1 chunks (of 8) streamed in e3m4; rest bf16
    n_q2=8,        # same for W2
    w1_bufs=8,
    w2_bufs=10,
    ht_bufs=2,
    os_bufs=2,
    ph_bufs=3,
    po_bufs=2,
    mm2_lag=1,     # consume chunk g-LAG behind mm1 emission
    w1_lead=5,     # w1 transfer stream leads w2 by this many chunks
    probe_no_mm1=0,  # timing diagnostics only: drop mm1 (keep DMAs)
    probe_no_mm2=0,  # timing diagnostics only: drop mm2 (keep DMAs)
    probe_no_store=0,  # timing diagnostics only: skip out stores
    ilv=1,         # interleave mm1/mm2 at half-chunk granularity
    pair_w1=0,     # DMA w1 chunks in pairs (halves trigger count); needs n_q1=8
    w1p_bufs=4,
    act_store=1,   # mid-expert out stores ride ACT HWDGE instead of Pool SWDGE
)

_CACHE = {}
LAST_RESULTS = None  # BassKernelResults of the most recent run (for profiling)
TPC = 4  # f-tiles per 512-wide chunk
N_CH = FT // TPC  # 8 chunks per expert


def _build_program(act="gelu", repeats=1, cfg=None):
    import contextlib

    import concourse.bacc as bacc
    import concourse.tile as tile
    import concourse.mybir as mybir

    cfg = dict(DEFAULT_CFG, **(cfg or {}))

    f32 = mybir.dt.float32
    fp16 = mybir.dt.float16
    bf16 = mybir.dt.bfloat16
    fp8 = mybir.dt.float8e3  # e3m4
    # CoreSim doesn't implement the Gelu LUTs; "tanh" is a sim-only stand-in
    # used by test.py to validate everything except the activation itself.
    GELU = {
        "gelu": mybir.ActivationFunctionType.Gelu_apprx_tanh,
        "tanh": mybir.ActivationFunctionType.Tanh,
    }[act]
    COPY = mybir.ActivationFunctionType.Copy
    IDENT = mybir.ActivationFunctionType.Identity

    nc = bacc.Bacc("TRN2", target_bir_lowering=False, debug=False)

    nq1, nq2 = cfg["n_q1"], cfg["n_q2"]
    assert 0 <= nq1 <= N_CH and 0 <= nq2 <= N_CH
    xT_d = nc.declare_dram_parameter("xT", [P, E_LOC, KT1, C], fp16, isOutput=False)
    # Weights arrive host-pre-tiled (and pre-scaled by SCALE) so every weight
    # DMA is one contiguous read of 4KB per partition:
    # w1[e, c, p, t, k, fc] = SCALE*W1[e, k*128+p, (c*4+t)*128+fc]
    # w2[e, c, p, t, j, dc] = SCALE*W2[e, (c*4+t)*128+p, j*128+dc]
    # The first nq chunks live in the e3m4 params, the rest in bf16 params.
    if cfg["pair_w1"]:
        assert nq1 == N_CH, "w1 pairing requires the all-e3m4 config"
    w_aps = {}
    for nm, nq in (("w1", nq1), ("w2", nq2)):
        q = h = None
        shp = [P, TPC, KT1, P]
        if nm == "w1" and cfg["pair_w1"]:
            # pair-major: one DMA brings two chunks (8KB/partition contiguous)
            q = nc.declare_dram_parameter(
                "w1q", [E_LOC, N_CH // 2, P, 2, TPC, KT1, P], fp8, isOutput=False
            ).ap()
        elif nq > 0:
            q = nc.declare_dram_parameter(
                nm + "q", [E_LOC, nq] + shp, fp8, isOutput=False
            ).ap()
        if nq < N_CH:
            h = nc.declare_dram_parameter(
                nm + "h", [E_LOC, N_CH - nq] + shp, bf16, isOutput=False
            ).ap()
        w_aps[nm] = (q, h, nq)
    b1_d = nc.declare_dram_parameter("b1t", [P, E_LOC, FT], f32, isOutput=False)
    b2_d = nc.declare_dram_parameter("b2t", [P, E_LOC, KT1], f32, isOutput=False)
    out_d = nc.declare_dram_parameter("out", [E_LOC, P, KT1, C], fp16, isOutput=True)

    with tile.TileContext(nc) as tc:
        with (
            tc.tile_pool(name="const", bufs=1) as const_pool,
            tc.tile_pool(name="w1", bufs=cfg["w1_bufs"]) as w1_pool,
            tc.tile_pool(name="w1p", bufs=cfg["w1p_bufs"]) as w1p_pool,
            tc.tile_pool(name="w2", bufs=cfg["w2_bufs"]) as w2_pool,
            tc.tile_pool(name="ht", bufs=cfg["ht_bufs"]) as ht_pool,
            tc.tile_pool(name="os", bufs=cfg["os_bufs"]) as os_pool,
            tc.tile_pool(name="ph", bufs=cfg["ph_bufs"], space="PSUM") as ph_pool,
            tc.tile_pool(name="po", bufs=cfg["po_bufs"], space="PSUM") as po_pool,
        ):
            pools = dict(
                w1=w1_pool, w1p=w1p_pool, w2=w2_pool, ht=ht_pool, os=os_pool,
                ph=ph_pool, po=po_pool,
            )
            # Consts ride the Pool/SWDGE queue so the SP queue's first w1
            # DMA is never delayed. Only xT[e0] gates the first matmul; b1
            # must land by the first gelu (~5.5us); b2t by the first expert
            # evacuation (~25us); xT[e>0] by expert e (~25us+).
            xT_sb = const_pool.tile([P, E_LOC, KT1, C], fp16, tag="xt")
            nc.gpsimd.dma_start(out=xT_sb[:, 0], in_=xT_d.ap()[:, 0])

            def late_consts():
                b1_sb = const_pool.tile([P, E_LOC, FT], f32, tag="b1")
                nc.gpsimd.dma_start(out=b1_sb, in_=b1_d.ap())
                b2_sb = const_pool.tile([P, E_LOC, KT1], f32, tag="b2")
                nc.gpsimd.dma_start(out=b2_sb, in_=b2_d.ap())
                for e in range(1, E_LOC):
                    nc.gpsimd.dma_start(out=xT_sb[:, e], in_=xT_d.ap()[:, e])
                return b1_sb, b2_sb

            consts = (xT_sb, late_consts)

            # repeats>1 wraps the computation in a hardware loop so a single
            # execute measures R back-to-back runs (benchmarking only).
            rep_ctx = (
                tc.For_i(0, repeats, 1) if repeats > 1 else contextlib.nullcontext()
            )
            with rep_ctx:
                _emit_body(
                    nc, GELU, IDENT, consts, w_aps, out_d, pools,
                    (f32, fp16, bf16, fp8), cfg,
                )

    nc.compile()
    return nc


def _emit_body(nc, GELU, IDENT, consts, w_aps, out_d, pools, dts, cfg):
    import concourse.mybir as mybir

    ALU_MULT = mybir.AluOpType.mult
    ALU_ADD = mybir.AluOpType.add
    xT_sb, late_consts = consts
    f32, fp16, bf16, fp8 = dts
    LAG = cfg["mm2_lag"]  # consume (mm2) chunk g-LAG while mm1 runs chunk g
    LEAD = cfg["w1_lead"]  # w1 DMA emission runs LEAD steps ahead of w2's
    INV = 1.0 / SCALE
    assert cfg["w1_bufs"] >= LEAD + 2 and cfg["w2_bufs"] >= LAG + 2
    assert cfg["ph_bufs"] >= LAG + 1 and cfg["po_bufs"] >= 2

    def src_of(which, e, s):
        q, h, nq = w_aps[which]
        return (q[e, s], fp8) if s < nq else (h[e, s - nq], bf16)

    SPE = N_CH  # pipeline steps per expert
    G = E_LOC * SPE
    state = {}  # e -> (hT, oT, done)
    w1q, w2q, phq = {}, {}, {}
    lc = [None]

    def issue_w1(g):
        if g >= G:
            return
        e, s = divmod(g, SPE)
        if cfg["pair_w1"]:
            # One DMA trigger brings a 2-chunk pair (8KB/partition); both
            # chunks' consumers read per-chunk views of the pair tile.
            # Expert 0's pair 0 still arrives as singles (+ the half-chunk
            # warmup split) so the first matmul isn't gated on 1MB.
            qp = w_aps["w1"][0]
            sp, half = divmod(s, 2)
            if e == 0 and sp == 0:
                if s == 0:
                    ha = pools["w1"].tile([P, TPC // 2, KT1, P], fp8, tag="w1a")
                    nc.sync.dma_start(out=ha, in_=qp[e, 0][:, 0, 0 : TPC // 2])
                    hb = pools["w1"].tile([P, TPC // 2, KT1, P], fp8, tag="w1a")
                    nc.sync.dma_start(out=hb, in_=qp[e, 0][:, 0, TPC // 2 : TPC])
                    w1q[0] = (ha, hb)
                else:
                    t1 = pools["w1"].tile([P, TPC, KT1, P], fp8, tag="w1s")
                    nc.sync.dma_start(out=t1, in_=qp[e, 0][:, 1])
                    w1q[1] = t1
                return
            if half == 0:
                tp_ = pools["w1p"].tile([P, 2, TPC, KT1, P], fp8, tag="w1p")
                nc.sync.dma_start(out=tp_, in_=qp[e, sp])
                w1q[g] = tp_[:, 0]
                w1q[g + 1] = tp_[:, 1]
            return
        src, dt = src_of("w1", e, s)
        if g == 0:
            # Warmup split: the first matmul only needs the first half-chunk
            # (256KB), so it starts ~0.7us earlier; chunk 0's mm1 runs
            # t-major so its first 16 matmuls touch only the first half.
            ha = pools["w1"].tile([P, TPC // 2, KT1, P], dt, tag="w1" + dt.name)
            nc.sync.dma_start(out=ha, in_=src[:, 0 : TPC // 2])
            hb = pools["w1"].tile([P, TPC // 2, KT1, P], dt, tag="w1" + dt.name)
            nc.sync.dma_start(out=hb, in_=src[:, TPC // 2 : TPC])
            w1q[g] = (ha, hb)
            return
        t = pools["w1"].tile([P, TPC, KT1, P], dt, tag="w1" + dt.name)
        nc.sync.dma_start(out=t, in_=src)
        w1q[g] = t

    def issue_w2(g):
        e, s = divmod(g, SPE)
        src, dt = src_of("w2", e, s)
        if e == E_LOC - 1 and s == SPE - 1:
            # Tail split 3+1: the final mm2's weight DMA shrinks to one
            # f-tile, so the work exposed after the last weight byte lands
            # is 8 short matmuls instead of 32.
            ta = pools["w2"].tile([P, TPC - 1, KT1, P], dt, tag="w2" + dt.name)
            nc.sync.dma_start(out=ta, in_=src[:, 0 : TPC - 1])
            tb = pools["w2"].tile([P, 1, KT1, P], dt, tag="w2b" + dt.name)
            nc.sync.dma_start(out=tb, in_=src[:, TPC - 1 : TPC])
            w2q[g] = (ta, tb)
        else:
            t = pools["w2"].tile([P, TPC, KT1, P], dt, tag="w2" + dt.name)
            nc.sync.dma_start(out=t, in_=src)
            w2q[g] = t

    def mm1(g, ks=None):
        e, s = divmod(g, SPE)
        if ks is None or ks[0] == 0:
            w1t = w1q.pop(g)
            hp = pools["ph"].tile([P, TPC, C], f32, tag="hp")
            mm1.cur = (w1t, hp)
        else:
            w1t, hp = mm1.cur
        b1_sb = lc[0][0]
        # One accumulation group per chunk: start=True zeroes the WHOLE 2KB
        # PSUM bank (the hardware zero region), so the 4 f-tiles of a chunk
        # must share a single group — first matmul starts it, last stops it,
        # every matmul accumulates its own [128, 64] slice of the bank.
        # k-major order interleaves the 4 f-tile slices (measured ~42 vs
        # 45.5 ns/matmul for straight chains) and reuses one xT[k] moving
        # operand for 4 consecutive matmuls. Chunk 0 runs t-major instead,
        # so its first 16 matmuls only need the first warmup half-DMA.
        if isinstance(w1t, tuple):
            order = [(k, t) for t in range(TPC) for k in range(KT1)]
        else:
            order = [(k, t) for k in range(KT1) for t in range(TPC)]
        if cfg["probe_no_mm1"]:
            order = [(k, t) for (k, t) in order if k == 0]
        first_kt, last_kt = order[0], order[-1]
        if ks is not None:
            order = [(k, t) for (k, t) in order if k in ks]
        for (k, t) in order:
            if isinstance(w1t, tuple):
                wt1, t1 = (w1t[0], t) if t < TPC // 2 else (w1t[1], t - TPC // 2)
            else:
                wt1, t1 = w1t, t
            nc.tensor.matmul(
                hp[:, t, :],
                lhsT=wt1[:, t1, k, :],
                rhs=xT_sb[:, e, k, :],
                start=((k, t) == first_kt),
                stop=((k, t) == last_kt),
            )
        if ks is not None and ks[-1] != KT1 - 1:
            return
        for t in range(TPC):
            ft = s * TPC + t
            # gelu reads the PSUM f-tile directly: per-partition bias b1[f],
            # scale folds the 1/64 weight unquant. ACT runs these while PE
            # moves on to the next chunk.
            nc.scalar.activation(
                out=state[e][0][:, ft, :], in_=hp[:, t, :], func=GELU,
                bias=b1_sb[:, e, ft : ft + 1], scale=INV,
            )
        phq[g] = hp

    def consume_m(g, ts=None):
        if g < 0:
            return
        e, s = divmod(g, SPE)
        hT, oT, done = state[e]
        b2_sb = lc[0][1]
        if ts is None or ts[0] == 0:
            w2t = w2q.pop(g)
            consume_m.cur = w2t
        else:
            w2t = consume_m.cur
        for t in (range(TPC) if ts is None else ts):
            if isinstance(w2t, tuple):
                wt, tt = (w2t[0], t) if t < TPC - 1 else (w2t[1], 0)
            else:
                wt, tt = w2t, t
            ft = s * TPC + t
            done[0] += 1
            first = done[0] == 1
            last = done[0] == FT
            if cfg["probe_no_mm2"] and not (first or last):
                continue
            # Like mm1, the 8 d-block slots share the expert's single PSUM
            # bank and therefore a single accumulation group: only the very
            # first matmul of the expert starts it, only the very last stops.
            for j in range(KT1):
                nc.tensor.matmul(
                    oT[:, j, :],
                    lhsT=wt[:, tt, j, :],
                    rhs=hT[:, ft, :],
                    start=(first and j == 0),
                    stop=(last and j == KT1 - 1),
                )
        if s == SPE - 1 and (ts is None or ts[-1] == TPC - 1):
            # Evacuation: per-d-block out = psum*(1/64) + b2[d] — the bias
            # rides the evacuation (DVE tensor_scalar mult+add / ACT Copy
            # with per-partition bias) instead of costing PE rank-1 matmuls.
            # DVE (otherwise idle in this dataflow) carries mid-stream
            # evacuations so ACT stays dedicated to gelus; out stores ride
            # Pool/SWDGE. The LAST expert splits halves across DVE + ACT
            # with stores on separate DGE units (Pool SWDGE + ACT HWDGE) so
            # the two tails drain in parallel — it is the kernel's critical
            # tail.
            os_t = pools["os"].tile([P, KT1, C], fp16, tag="os")
            orow = out_d.ap()[e]
            last_e = e == E_LOC - 1
            half = KT1 // 2 if last_e else KT1
            for j in range(half):
                nc.vector.tensor_scalar(
                    out=os_t[:, j, :], in0=oT[:, j, :],
                    scalar1=INV, scalar2=b2_sb[:, e, j : j + 1],
                    op0=ALU_MULT, op1=ALU_ADD,
                )
            if not cfg["probe_no_store"]:
                # Mid-expert stores: ACT HWDGE trigger (0.63us on the idle
                # HWDGE unit) instead of Pool SWDGE (1us serial on the Pool
                # engine); the trigger's DVE-evac wait resolves well before
                # the next expert's first gelu needs the ACT queue. The
                # last expert's first half keeps Pool SWDGE so its two tail
                # stores drain on separate DGE units.
                if last_e or not cfg["act_store"]:
                    nc.gpsimd.dma_start(out=orow[:, 0:half], in_=os_t[:, 0:half, :])
                else:
                    nc.scalar.dma_start(out=orow[:, 0:half], in_=os_t[:, 0:half, :])
            if last_e and not cfg["probe_no_store"]:
                # ACT's compute must never touch the evacuation: any
                # non-Gelu activation func would force a ~1.3us LUT table
                # reload on the critical tail (twice per repeat). DVE
                # finishes the second half while the first half's store is
                # already in flight; the second store is only a DMA TRIGGER
                # on the ACT queue (HWDGE, no LUT) — NOT on the SP queue,
                # whose in-order head would block the next repeat
                # iteration's weight DMAs behind this late-waiting store.
                for j in range(half, KT1):
                    nc.vector.tensor_scalar(
                        out=os_t[:, j, :], in0=oT[:, j, :],
                        scalar1=INV, scalar2=b2_sb[:, e, j : j + 1],
                        op0=ALU_MULT, op1=ALU_ADD,
                    )
                nc.scalar.dma_start(out=orow[:, half:KT1], in_=os_t[:, half:KT1, :])
            del state[e]

    for g in range(LEAD):
        issue_w1(g)
    lc[0] = late_consts()
    for g in range(G):
        e, s = divmod(g, SPE)
        if s == 0:
            hT = pools["ht"].tile([P, FT, C], fp16, tag="ht")
            oT = pools["po"].tile([P, KT1, C], f32, tag="ot")
            state[e] = (hT, oT, [0])
        issue_w1(g + LEAD)
        issue_w2(g)
        if cfg["ilv"] and g - LAG >= 0 and g > 0:
            mm1(g, ks=(0, 1, 2, 3))
            consume_m(g - LAG, ts=(0, 1))
            mm1(g, ks=(4, 5, 6, 7))
            consume_m(g - LAG, ts=(2, 3))
        else:
            mm1(g)
            consume_m(g - LAG)
        if g - LAG - 1 >= 0:
            phq.pop(g - LAG - 1, None)
    for g in range(G - LAG, G):
        consume_m(g)


def _get_program(act="gelu", repeats=1, cfg=None):
    key = (act, repeats, tuple(sorted((cfg or {}).items())))
    if key not in _CACHE:
        _CACHE[key] = _build_program(act, repeats, cfg)
    return _CACHE[key]


def make_in_maps(x, W1, b1, W2, b2, cfg=None):
    import ml_dtypes

    bf16 = ml_dtypes.bfloat16
    fp8 = ml_dtypes.float8_e3m4
    fp16 = np.float16
    cfg = dict(DEFAULT_CFG, **(cfg or {}))
    nq1, nq2 = cfg["n_q1"], cfg["n_q2"]
    x = np.ascontiguousarray(np.asarray(x, dtype=np.float32))
    W1 = np.asarray(W1, dtype=np.float32)
    b1 = np.ascontiguousarray(np.asarray(b1, dtype=np.float32))
    W2 = np.asarray(W2, dtype=np.float32)
    b2 = np.ascontiguousarray(np.asarray(b2, dtype=np.float32))
    in_maps = []
    for i in range(N_CORES):
        lo, hi = i * E_LOC, (i + 1) * E_LOC
        xc = x[0, lo * C : hi * C, :].reshape(E_LOC, C, KT1, P)
        xT = np.ascontiguousarray(xc.transpose(3, 0, 2, 1)).astype(fp16)  # [128,e,k,c]
        b1t = np.ascontiguousarray(
            b1[lo:hi].reshape(E_LOC, FT, P).transpose(2, 0, 1)
        )  # [128, e, ft]
        b2t = np.ascontiguousarray(
            b2[lo:hi].reshape(E_LOC, KT1, P).transpose(2, 0, 1)
        )  # [128, e, j]  (unscaled: added after the 1/SCALE evacuation mult)
        # [e, chunk, p, t, k/j, 128], pre-scaled by SCALE (exact in bf16 too);
        # first nq chunks e3m4, rest bf16.
        w1full = (W1[lo:hi] * SCALE).reshape(E_LOC, KT1, P, N_CH, TPC, P)
        w1full = w1full.transpose(0, 3, 2, 4, 1, 5)
        w2full = (W2[lo:hi] * SCALE).reshape(E_LOC, N_CH, TPC, P, KT1, P)
        w2full = w2full.transpose(0, 1, 3, 2, 4, 5)
        m = {
            "xT": xT,
            "b1t": b1t,
            "b2t": b2t,
        }
        if cfg["pair_w1"]:
            m["w1q"] = np.ascontiguousarray(
                w1full.reshape(E_LOC, N_CH // 2, 2, P, TPC, KT1, P)
                .transpose(0, 1, 3, 2, 4, 5, 6)
            ).astype(fp8)
        elif nq1 > 0:
            m["w1q"] = np.ascontiguousarray(w1full[:, :nq1]).astype(fp8)
        if nq1 < N_CH:
            m["w1h"] = np.ascontiguousarray(w1full[:, nq1:]).astype(bf16)
        if nq2 > 0:
            m["w2q"] = np.ascontiguousarray(w2full[:, :nq2]).astype(fp8)
        if nq2 < N_CH:
            m["w2h"] = np.ascontiguousarray(w2full[:, nq2:]).astype(bf16)
        in_maps.append(m)
    return in_maps


def unshuffle_out(out_t):
    """[E_LOC, 128p, 8j, 64c] (transposed d-major device layout) ->
    [E_LOC*C, D] with out[e*64+c, j*128+p]."""
    return np.ascontiguousarray(
        np.asarray(out_t).transpose(0, 3, 2, 1)
    ).reshape(E_LOC * C, D)


def kernel(x, W1, b1, W2, b2):
    global LAST_RESULTS
    from concourse.bass_utils import run_bass_kernel_spmd

    nc = _get_program()
    in_maps = make_in_maps(x, W1, b1, W2, b2)
    trace = bool(int(os.environ.get("KERNEL_TRACE", "0")))
    res = run_bass_kernel_spmd(nc, in_maps, list(range(N_CORES)), trace=trace)
    LAST_RESULTS = res
    out = np.concatenate([unshuffle_out(r["out"]) for r in res.results], axis=0)
    return out.reshape(1, E * C, D).astype(np.float32)
